# revision 13
# baseline (speedup 1.0000x reference)
"""Trainium2 Bass kernel for nn_CkConv1D (continuous-kernel causal conv).

Math: the reference builds a T x T Toeplitz kernel K[o,c,i,j] =
sum_h w2[h]*sin(A_h*(j-i) + off[o,c,h]) + b2  (A_h = w1[h,0]/T), masks it
causally (j<=i) and contracts with x.  Using sin(X+Y) = sinX cosY + cosX sinY
with X = A_h*j, Y = off - A_h*i, the masked contraction factorizes into
causal prefix sums over j of sin(A_h j)x[j,c] / cos(A_h j)x[j,c].

Work is sharded over 8 NeuronCores: core m produces output rows
[128m, 128m+128).  The host precomputes every weight-only quantity (the
trig basis over j, the per-core window trig, the w2-scaled query-side
trig) so the device does only the x-dependent contractions:

  R[jj,(s,c,h)]   = TW[jj,s,h] * xwin[jj,c]                  (DVE)
  pwS/pwC[p,ii]   = R_s.T @ ut   (windowed causal prefix)    (PE, bf16)
  colT[(s,h),c]   = sum_b TT_b.T @ xm_b  (block prefix)      (PE, bf16)
  G_s             = pwS * QTc'   (window part)               (DVE)
  y1              = sum_c colT_c.T @ QTY_c                   (PE, rank-1
                    + ones.T @ G_s + ones.T @ G_c             block-prefix
  y               = y1 + b2 * (causal prefix of sum_c x)      fold-in)

The block-prefix correction never needs a partition transpose: its
contribution to y is linear in colT, so it folds into the final PSUM
accumulation as four rank-1 matmuls against the host-built QTY grid.

Partition layout: p = c*32 + h (C_in=4 channels x H=32 hidden = 128).
The program is identical on every core (SPMD); per-core behavior comes
only from per-core input data.
"""

import sys
from pathlib import Path

import numpy as np

for _p in ("/opt/trn_rl_repo",):
    if _p not in sys.path and Path(_p).exists():
        sys.path.insert(0, _p)

import ml_dtypes
import concourse.bass as bass
import concourse.bacc as bacc
import concourse.tile as tile
from concourse import mybir
from concourse.bass_utils import run_bass_kernel_spmd

F32 = mybir.dt.float32
BF16 = mybir.dt.bfloat16
T, C, O, H, P, M = 1024, 4, 2, 32, 128, 8

# big (bf16) column offsets
TW_OFF = 0            # window trig [jj, (s, h)]            (64)
XW_OFF = 64           # window x  [jj, c]                   (4)
ONES_OFF = 68         # ones column                         (1)
UT_OFF = 69           # ut[jj, ii] = 1 if jj <= ii          (128)
XM_OFF = 197          # masked x  [jj, (b, c)]              (32)
TT_OFF = 229          # basis trig [jj, (b, s, h)]          (512)
NB = 741

# qt (fp32): query-side trig
QTS_OFF = 0           # w2[h]*sin(off - A_h*i)  [p, (o, ii)]  (256)
QTC_OFF = 256         # w2[h]*cos(off - A_h*i)  [p, (o, ii)]  (256)
B2_OFF = 512          # b2 replicated                          (1)
NQ = 513

NY = C * O * P        # qty (bf16) [64, (c, o, ii)]

_nc_cache = {}


def _build_nc():
    nc = bacc.Bacc()
    big_d = nc.dram_tensor("big", [P, NB], BF16, kind="ExternalInput")
    qt_d = nc.dram_tensor("qt", [P, NQ], F32, kind="ExternalInput")
    qty_d = nc.dram_tensor("qty", [2 * H, NY], BF16, kind="ExternalInput")
    y_d = nc.dram_tensor("y", [1, O, P], F32, kind="ExternalOutput")

    Mult = mybir.AluOpType.mult
    Add = mybir.AluOpType.add
    AxX = mybir.AxisListType.X

    with tile.TileContext(nc) as tc:
        with (
            tc.tile_pool(name="sb", bufs=1) as sb,
            tc.tile_pool(name="ps", bufs=1, space="PSUM") as ps,
        ):
            big = sb.tile([P, NB], BF16)
            qt = sb.tile([P, NQ], F32)
            qty = sb.tile([2 * H, NY], BF16)
            nc.sync.dma_start(out=big[:], in_=big_d[:])
            nc.scalar.dma_start(out=qt[:], in_=qt_d[:])
            nc.gpsimd.dma_start(out=qty[:], in_=qty_d[:])

            tw = big[:, TW_OFF:TW_OFF + 2 * H].rearrange(
                "p (s h) -> p s h", s=2)
            xwin = big[:, XW_OFF:XW_OFF + C]
            ones = big[:, ONES_OFF:ONES_OFF + 1]
            ut = big[:, UT_OFF:UT_OFF + P]
            xm = big[:, XM_OFF:XM_OFF + M * C].rearrange(
                "p (b c) -> p b c", b=M)
            xmf = big[:, XM_OFF:XM_OFF + M * C]
            tt = big[:, TT_OFF:TT_OFF + 2 * M * H].rearrange(
                "p (b s h) -> p b s h", b=M, s=2)
            qts = qt[:, QTS_OFF:QTS_OFF + O * P].rearrange(
                "p (o i) -> p o i", o=O)
            qtc = qt[:, QTC_OFF:QTC_OFF + O * P].rearrange(
                "p (o i) -> p o i", o=O)
            b2col = qt[0:1, B2_OFF:B2_OFF + 1]
            qty_v = qty[:].rearrange("p (c n) -> p c n", c=C)

            # ---- window products R[jj, (s, c, h)] = TW[jj,s,h]*xwin[jj,c]
            R = sb.tile([P, 2, C, H], BF16)
            tw_b = tw.unsqueeze(2).broadcast_to([P, 2, C, H])
            xw_b = xwin.unsqueeze(1).unsqueeze(3).broadcast_to([P, 2, C, H])
            nc.vector.tensor_mul(R[:], tw_b, xw_b)

            # ---- row sums for the b2 term
            srow = sb.tile([P, 2], BF16)
            with nc.allow_low_precision(reason="4/32-term bf16 row sums"):
                nc.vector.reduce_sum(srow[:, 0:1], xwin, axis=AxX)
                nc.vector.reduce_sum(srow[:, 1:2], xmf, axis=AxX)

            # ---- PE contractions ----
            # block prefix: colT[(s,h), c] = sum_b TT_b.T @ xm_b
            colT = ps.tile([2 * H, C], F32)
            for b in range(M):
                nc.tensor.matmul(colT[:], tt[:, b], xm[:, b, :],
                                 start=(b == 0), stop=(b == M - 1))
            # windowed causal prefixes
            pwS = ps.tile([P, P], F32)
            pwC = ps.tile([P, P], F32)
            nc.tensor.matmul(pwS[:], R[:, 0], ut, start=True, stop=True)
            nc.tensor.matmul(pwC[:], R[:, 1], ut, start=True, stop=True)
            # x prefix for the b2 term: pwx[0, ii] = window prefix + block sum
            pwx = ps.tile([1, P], F32)
            nc.tensor.matmul(pwx[:], srow[:, 0:1], ut, start=True, stop=False)
            nc.tensor.matmul(pwx[:], srow[:, 1:2],
                             ones.broadcast_to([P, P]), start=False, stop=True)

            # ---- drain colT/pwx from PSUM on the (idle) ACT engine ----
            colT_sb = sb.tile([2 * H, C], BF16)
            nc.scalar.copy(colT_sb[:], colT[:])
            pwx_sb = sb.tile([1, P], F32)
            nc.scalar.copy(pwx_sb[:], pwx[:])

            # ---- combine G_s = pwS*QTc', G_c = pwC*QTs' (no col dep) ----
            G = sb.tile([P, 2, O, P], BF16)
            pwS_b = pwS[:].unsqueeze(1).broadcast_to([P, O, P])
            pwC_b = pwC[:].unsqueeze(1).broadcast_to([P, O, P])
            nc.vector.tensor_mul(G[:, 0], pwS_b, qtc)
            nc.vector.tensor_mul(G[:, 1], pwC_b, qts)

            # ---- final projection into one PSUM accumulation group ----
            # y1 = sum_c colT_c.T @ QTY_c  (block prefix)  +  sum_p G
            y1 = ps.tile([1, O, P], F32)
            for c in range(C):
                nc.tensor.matmul(y1[:], colT_sb[:, c:c + 1], qty_v[:, c],
                                 start=(c == 0), stop=False,
                                 skip_group_check=True)
            nc.tensor.matmul(y1[:], ones, G[:, 0].rearrange("p o i -> p (o i)"),
                             start=False, stop=False, skip_group_check=True)
            nc.tensor.matmul(y1[:], ones, G[:, 1].rearrange("p o i -> p (o i)"),
                             start=False, stop=True, skip_group_check=True)

            # ---- y = b2 * pwx + y1 ----
            ysb = sb.tile([1, O, P], F32)
            pwx_b = pwx_sb[:].unsqueeze(1).broadcast_to([1, O, P])
            nc.vector.scalar_tensor_tensor(ysb[:], pwx_b, b2col, y1[:],
                                           Mult, Add)
            nc.sync.dma_start(out=y_d[:], in_=ysb[:])
    nc.finalize()
    return nc


def _host_inputs(x, w1, b1, w2, b2):
    """Per-core input maps.  Host precomputes all weight-only trig."""
    x = np.asarray(x, np.float64)
    w1 = np.asarray(w1, np.float64)
    b1 = np.asarray(b1, np.float64)
    w2 = np.asarray(w2, np.float64)
    b2 = np.asarray(b2, np.float64)

    A = w1[:, 0] / T                                   # [H]
    jj = np.arange(P)
    bb = np.arange(M)
    ang = A[None, None, :] * (P * bb[None, :, None] + jj[:, None, None])
    ttfull = np.stack([np.sin(ang), np.cos(ang)], axis=1)  # [jj, s, b, h]
    ut = np.triu(np.ones((P, P)))
    cc = np.arange(C)
    oo = np.arange(O)
    off = (oo[:, None, None] * w1[:, 2]
           + cc[None, :, None] * w1[:, 1] + b1)        # [o, c, h]

    xr = x.reshape(M, P, C)
    in_maps = []
    for m in range(M):
        i_vals = P * m + jj                            # [ii]
        q = off[:, :, :, None] - A[None, None, :, None] * i_vals  # [o,c,h,ii]
        qts = (w2[0][None, :, None, None] * np.sin(q).transpose(1, 2, 0, 3)
               ).reshape(P, O * P)                     # [p=(c,h), (o,ii)]
        qtc = (w2[0][None, :, None, None] * np.cos(q).transpose(1, 2, 0, 3)
               ).reshape(P, O * P)
        xmask = x.copy()
        xmask[P * m:] = 0.0
        xm = xmask.reshape(M, P, C).transpose(1, 0, 2).reshape(P, M * C)

        big = np.zeros((P, NB), np.float64)
        big[:, TW_OFF:TW_OFF + 2 * H] = ttfull[:, :, m, :].reshape(P, 2 * H)
        big[:, XW_OFF:XW_OFF + C] = xr[m]
        big[:, ONES_OFF] = 1.0
        big[:, UT_OFF:UT_OFF + P] = ut
        big[:, XM_OFF:XM_OFF + M * C] = xm
        big[:, TT_OFF:TT_OFF + 2 * M * H] = ttfull.transpose(
            0, 2, 1, 3).reshape(P, 2 * M * H)          # [jj, (b, s, h)]

        qtf = np.zeros((P, NQ), np.float32)
        qtf[:, QTS_OFF:QTS_OFF + O * P] = qts
        qtf[:, QTC_OFF:QTC_OFF + O * P] = qtc
        qtf[:, B2_OFF] = b2[0]

        # qty[(s', h), (c, o, ii)]: s'=0 pairs col_s with QTc', s'=1 with QTs'
        qtyf = np.empty((2 * H, C, O * P), np.float64)
        qtyf[0:H] = qtc.reshape(C, H, O * P).transpose(1, 0, 2)
        qtyf[H:2 * H] = qts.reshape(C, H, O * P).transpose(1, 0, 2)
        qtyf = qtyf.reshape(2 * H, NY)

        in_maps.append({
            "big": big.astype(ml_dtypes.bfloat16),
            "qt": qtf,
            "qty": qtyf.astype(ml_dtypes.bfloat16),
        })
    return in_maps


def kernel(x, t, w1, b1, w2, b2, out_channels):
    if "nc" not in _nc_cache:
        _nc_cache["nc"] = _build_nc()
    nc = _nc_cache["nc"]
    in_maps = _host_inputs(x, w1, b1, w2, b2)
    res = run_bass_kernel_spmd(nc, in_maps, core_ids=list(range(M)))
    y = np.empty((T, O), np.float32)
    for m in range(M):
        ym = np.asarray(res.results[m]["y"]).reshape(O, P)
        y[P * m:P * (m + 1), :] = ym.T
    return y


# revision 14
# speedup vs baseline: 1.1119x; 1.1119x over previous
"""Trainium2 Bass kernel for nn_CkConv1D (continuous-kernel causal conv).

Math: the reference builds a T x T Toeplitz kernel K[o,c,i,j] =
sum_h w2[h]*sin(A_h*(j-i) + off[o,c,h]) + b2  (A_h = w1[h,0]/T), masks it
causally (j<=i) and contracts with x.  Using sin(X+Y) = sinX cosY + cosX sinY
with X = A_h*j, Y = off - A_h*i, the masked contraction factorizes into
causal prefix sums over j of sin(A_h j)x[j,c] / cos(A_h j)x[j,c].

Work is sharded over 8 NeuronCores: core m produces output rows
[128m, 128m+128).  The host precomputes every weight-only quantity (the
trig basis over j, the per-core window trig, the w2-scaled query-side
trig) so the device does only the x-dependent contractions:

  R[jj,(s,c,h)]   = TW[jj,s,h] * xwin[jj,c]                  (DVE)
  pwS/pwC[p,ii]   = R_s.T @ ut   (windowed causal prefix)    (PE, bf16)
  colT[(s,h),c]   = sum_b TT_b.T @ xm_b  (block prefix)      (PE, bf16)
  G_s             = pwS * QTc'   (window part)               (DVE)
  y1              = sum_c colT_c.T @ QTY_c                   (PE, rank-1
                    + ones.T @ G_s + ones.T @ G_c             block-prefix
  y               = y1 + causal prefix of b2*sum_c x          fold-in)

The block-prefix correction never needs a partition transpose: its
contribution to y is linear in colT, so it folds into the final PSUM
accumulation as four rank-1 matmuls against the host-built QTY grid.
The DMA phase is aggregate-HBM-bound across the 8 cores, so all device
constants travel in a single bf16 tensor (plus QTY), and b2 rides as
pre-scaled copies of the x row sums instead of an fp32 scalar.

Partition layout: p = c*32 + h (C_in=4 channels x H=32 hidden = 128).
The program is identical on every core (SPMD); per-core behavior comes
only from per-core input data.
"""

import sys
from pathlib import Path

import numpy as np

for _p in ("/opt/trn_rl_repo",):
    if _p not in sys.path and Path(_p).exists():
        sys.path.insert(0, _p)

import ml_dtypes
import concourse.bass as bass
import concourse.bacc as bacc
import concourse.tile as tile
from concourse import mybir
from concourse.bass_utils import run_bass_kernel_spmd

F32 = mybir.dt.float32
BF16 = mybir.dt.bfloat16
T, C, O, H, P, M = 1024, 4, 2, 32, 128, 8

# big (bf16) column offsets
TW_OFF = 0            # window trig [jj, (s, h)]            (64)
XW_OFF = 64           # window x  [jj, c]                   (4)
ONES_OFF = 68         # ones column                         (1)
UT_OFF = 69           # ut[jj, ii] = 1 if jj <= ii          (128)
XM_OFF = 197          # masked x  [jj, (b, c)]              (32)
TT_OFF = 229          # basis trig [jj, (b, s, h)]          (512)
QTS_OFF = 741         # w2[h]*sin(off - A_h*i) [p, (o,ii)]  (256)
QTC_OFF = 997         # w2[h]*cos(off - A_h*i) [p, (o,ii)]  (256)
XW2_OFF = 1253        # b2 * xwin                           (4)
XM2_OFF = 1257        # b2 * xm                             (32)
NB = 1289

NY = C * O * P        # qty (bf16) [64, (c, o, ii)]

_nc_cache = {}


def _build_nc():
    nc = bacc.Bacc()
    big_d = nc.dram_tensor("big", [P, NB], BF16, kind="ExternalInput")
    qty_d = nc.dram_tensor("qty", [2 * H, NY], BF16, kind="ExternalInput")
    y_d = nc.dram_tensor("y", [1, O, P], F32, kind="ExternalOutput")

    AxX = mybir.AxisListType.X

    with tile.TileContext(nc) as tc:
        with (
            tc.tile_pool(name="sb", bufs=1) as sb,
            tc.tile_pool(name="ps", bufs=1, space="PSUM") as ps,
        ):
            big = sb.tile([P, NB], BF16)
            qty = sb.tile([2 * H, NY], BF16)
            nc.sync.dma_start(out=big[:], in_=big_d[:])
            nc.scalar.dma_start(out=qty[:], in_=qty_d[:])

            tw = big[:, TW_OFF:TW_OFF + 2 * H].rearrange(
                "p (s h) -> p s h", s=2)
            xwin = big[:, XW_OFF:XW_OFF + C]
            ones = big[:, ONES_OFF:ONES_OFF + 1]
            ut = big[:, UT_OFF:UT_OFF + P]
            xm = big[:, XM_OFF:XM_OFF + M * C].rearrange(
                "p (b c) -> p b c", b=M)
            tt = big[:, TT_OFF:TT_OFF + 2 * M * H].rearrange(
                "p (b s h) -> p b s h", b=M, s=2)
            qts = big[:, QTS_OFF:QTS_OFF + O * P].rearrange(
                "p (o i) -> p o i", o=O)
            qtc = big[:, QTC_OFF:QTC_OFF + O * P].rearrange(
                "p (o i) -> p o i", o=O)
            xw2 = big[:, XW2_OFF:XW2_OFF + C]
            xm2 = big[:, XM2_OFF:XM2_OFF + M * C]
            qty_v = qty[:].rearrange("p (c n) -> p c n", c=C)

            # ---- window products R[jj, (s, c, h)] = TW[jj,s,h]*xwin[jj,c]
            R = sb.tile([P, 2, C, H], BF16)
            tw_b = tw.unsqueeze(2).broadcast_to([P, 2, C, H])
            xw_b = xwin.unsqueeze(1).unsqueeze(3).broadcast_to([P, 2, C, H])
            nc.vector.tensor_mul(R[:], tw_b, xw_b)

            # ---- b2-scaled row sums for the bias term
            srow = sb.tile([P, 2], BF16)
            with nc.allow_low_precision(reason="4/32-term bf16 row sums"):
                nc.vector.reduce_sum(srow[:, 0:1], xw2, axis=AxX)
                nc.vector.reduce_sum(srow[:, 1:2], xm2, axis=AxX)

            # ---- PE contractions ----
            # block prefix: colT[(s,h), c] = sum_b TT_b.T @ xm_b
            colT = ps.tile([2 * H, C], F32)
            for b in range(M):
                nc.tensor.matmul(colT[:], tt[:, b], xm[:, b, :],
                                 start=(b == 0), stop=(b == M - 1))
            # windowed causal prefixes
            pwS = ps.tile([P, P], F32)
            pwC = ps.tile([P, P], F32)
            nc.tensor.matmul(pwS[:], R[:, 0], ut, start=True, stop=True)
            nc.tensor.matmul(pwC[:], R[:, 1], ut, start=True, stop=True)
            # bias prefix: pwx[0, ii] = b2 * (window prefix + block sum) of x
            pwx = ps.tile([1, P], F32)
            nc.tensor.matmul(pwx[:], srow[:, 0:1], ut, start=True, stop=False)
            nc.tensor.matmul(pwx[:], srow[:, 1:2],
                             ones.broadcast_to([P, P]), start=False, stop=True)

            # ---- drain colT/pwx from PSUM on the (idle) ACT engine ----
            colT_sb = sb.tile([2 * H, C], BF16)
            nc.scalar.copy(colT_sb[:], colT[:])
            pwx_sb = sb.tile([1, P], F32)
            nc.scalar.copy(pwx_sb[:], pwx[:])

            # ---- combine G_s = pwS*QTc', G_c = pwC*QTs' (no col dep) ----
            G = sb.tile([P, 2, O, P], BF16)
            pwS_b = pwS[:].unsqueeze(1).broadcast_to([P, O, P])
            pwC_b = pwC[:].unsqueeze(1).broadcast_to([P, O, P])
            nc.vector.tensor_mul(G[:, 0], pwS_b, qtc)
            nc.vector.tensor_mul(G[:, 1], pwC_b, qts)

            # ---- final projection into one PSUM accumulation group ----
            # y1 = sum_c colT_c.T @ QTY_c  (block prefix)  +  sum_p G
            y1 = ps.tile([1, O, P], F32)
            for c in range(C):
                nc.tensor.matmul(y1[:], colT_sb[:, c:c + 1], qty_v[:, c],
                                 start=(c == 0), stop=False,
                                 skip_group_check=True)
            nc.tensor.matmul(y1[:], ones, G[:, 0].rearrange("p o i -> p (o i)"),
                             start=False, stop=False, skip_group_check=True)
            nc.tensor.matmul(y1[:], ones, G[:, 1].rearrange("p o i -> p (o i)"),
                             start=False, stop=True, skip_group_check=True)

            # ---- y = pwx + y1 ----
            ysb = sb.tile([1, O, P], F32)
            pwx_b = pwx_sb[:].unsqueeze(1).broadcast_to([1, O, P])
            nc.vector.tensor_add(ysb[:], pwx_b, y1[:])
            nc.sync.dma_start(out=y_d[:], in_=ysb[:])
    nc.finalize()
    return nc


def _host_inputs(x, w1, b1, w2, b2):
    """Per-core input maps.  Host precomputes all weight-only trig."""
    x = np.asarray(x, np.float64)
    w1 = np.asarray(w1, np.float64)
    b1 = np.asarray(b1, np.float64)
    w2 = np.asarray(w2, np.float64)
    b2 = np.asarray(b2, np.float64)

    A = w1[:, 0] / T                                   # [H]
    jj = np.arange(P)
    bb = np.arange(M)
    ang = A[None, None, :] * (P * bb[None, :, None] + jj[:, None, None])
    ttfull = np.stack([np.sin(ang), np.cos(ang)], axis=1)  # [jj, s, b, h]
    ut = np.triu(np.ones((P, P)))
    cc = np.arange(C)
    oo = np.arange(O)
    off = (oo[:, None, None] * w1[:, 2]
           + cc[None, :, None] * w1[:, 1] + b1)        # [o, c, h]

    xr = x.reshape(M, P, C)
    in_maps = []
    for m in range(M):
        i_vals = P * m + jj                            # [ii]
        q = off[:, :, :, None] - A[None, None, :, None] * i_vals  # [o,c,h,ii]
        qts = (w2[0][None, :, None, None] * np.sin(q).transpose(1, 2, 0, 3)
               ).reshape(P, O * P)                     # [p=(c,h), (o,ii)]
        qtc = (w2[0][None, :, None, None] * np.cos(q).transpose(1, 2, 0, 3)
               ).reshape(P, O * P)
        xmask = x.copy()
        xmask[P * m:] = 0.0
        xm = xmask.reshape(M, P, C).transpose(1, 0, 2).reshape(P, M * C)

        big = np.zeros((P, NB), np.float64)
        big[:, TW_OFF:TW_OFF + 2 * H] = ttfull[:, :, m, :].reshape(P, 2 * H)
        big[:, XW_OFF:XW_OFF + C] = xr[m]
        big[:, ONES_OFF] = 1.0
        big[:, UT_OFF:UT_OFF + P] = ut
        big[:, XM_OFF:XM_OFF + M * C] = xm
        big[:, TT_OFF:TT_OFF + 2 * M * H] = ttfull.transpose(
            0, 2, 1, 3).reshape(P, 2 * M * H)          # [jj, (b, s, h)]
        big[:, QTS_OFF:QTS_OFF + O * P] = qts
        big[:, QTC_OFF:QTC_OFF + O * P] = qtc
        big[:, XW2_OFF:XW2_OFF + C] = b2[0] * xr[m]
        big[:, XM2_OFF:XM2_OFF + M * C] = b2[0] * xm

        # qty[(s', h), (c, o, ii)]: s'=0 pairs col_s with QTc', s'=1 with QTs'
        qtyf = np.empty((2 * H, C, O * P), np.float64)
        qtyf[0:H] = qtc.reshape(C, H, O * P).transpose(1, 0, 2)
        qtyf[H:2 * H] = qts.reshape(C, H, O * P).transpose(1, 0, 2)
        qtyf = qtyf.reshape(2 * H, NY)

        in_maps.append({
            "big": big.astype(ml_dtypes.bfloat16),
            "qty": qtyf.astype(ml_dtypes.bfloat16),
        })
    return in_maps


def kernel(x, t, w1, b1, w2, b2, out_channels):
    if "nc" not in _nc_cache:
        _nc_cache["nc"] = _build_nc()
    nc = _nc_cache["nc"]
    in_maps = _host_inputs(x, w1, b1, w2, b2)
    res = run_bass_kernel_spmd(nc, in_maps, core_ids=list(range(M)))
    y = np.empty((T, O), np.float32)
    for m in range(M):
        ym = np.asarray(res.results[m]["y"]).reshape(O, P)
        y[P * m:P * (m + 1), :] = ym.T
    return y


# revision 15
# speedup vs baseline: 1.1229x; 1.0099x over previous
"""Trainium2 Bass kernel for nn_CkConv1D (continuous-kernel causal conv).

Math: the reference builds a T x T Toeplitz kernel K[o,c,i,j] =
sum_h w2[h]*sin(A_h*(j-i) + off[o,c,h]) + b2  (A_h = w1[h,0]/T), masks it
causally (j<=i) and contracts with x.  Using sin(X+Y) = sinX cosY + cosX sinY
with X = A_h*j, Y = off - A_h*i, the masked contraction factorizes into
causal prefix sums over j of sin(A_h j)x[j,c] / cos(A_h j)x[j,c].

Work is sharded over 8 NeuronCores: core m produces output rows
[128m, 128m+128).  The host precomputes every weight-only quantity (the
trig basis over j, the per-core window trig, the w2-scaled query-side
trig) so the device does only the x-dependent contractions:

  ut[jj,ii]       = causal mask, built by POOL affine_select
                    while the input DMAs are still in flight
  R[jj,(s,c,h)]   = TW[jj,s,h] * xwin[jj,c]                  (DVE)
  pwS/pwC[p,ii]   = R_s.T @ ut   (windowed causal prefix)    (PE, bf16)
  colT[(s,h),c]   = sum_b TT_b.T @ xm_b  (block prefix)      (PE, bf16)
  G_so            = pw_s * QT_o'  (4 quarters, pipelined     (DVE)
  y1              = sum_c colT_c.T @ QTY_c                    with PE)
                    + ones.T @ G_so   (one PSUM accumulation group)
  y               = y1 + causal prefix of b2*sum_c x

The block-prefix correction never needs a partition transpose: its
contribution to y is linear in colT, so it folds into the final PSUM
accumulation as four rank-1 matmuls against the host-built QTY grid.
The DMA phase is aggregate-HBM-bound across the 8 cores, so all device
constants travel in a single bf16 tensor (plus QTY), and b2 rides as
pre-scaled copies of the x row sums instead of an fp32 scalar.

Partition layout: p = c*32 + h (C_in=4 channels x H=32 hidden = 128).
The program is identical on every core (SPMD); per-core behavior comes
only from per-core input data.
"""

import sys
from pathlib import Path

import numpy as np

for _p in ("/opt/trn_rl_repo",):
    if _p not in sys.path and Path(_p).exists():
        sys.path.insert(0, _p)

import ml_dtypes
import concourse.bass as bass
import concourse.bacc as bacc
import concourse.tile as tile
from concourse import mybir
from concourse.bass_utils import run_bass_kernel_spmd

F32 = mybir.dt.float32
BF16 = mybir.dt.bfloat16
T, C, O, H, P, M = 1024, 4, 2, 32, 128, 8

# big (bf16) column offsets
TW_OFF = 0            # window trig [jj, (s, h)]            (64)
XW_OFF = 64           # window x  [jj, c]                   (4)
XM_OFF = 68           # masked x  [jj, (b, c)]              (32)
TT_OFF = 100          # basis trig [jj, (b, s, h)]          (512)
QTS_OFF = 612         # w2[h]*sin(off - A_h*i) [p, (o,ii)]  (256)
QTC_OFF = 868         # w2[h]*cos(off - A_h*i) [p, (o,ii)]  (256)
XW2_OFF = 1124        # b2 * xwin                           (4)
XM2_OFF = 1128        # b2 * xm                             (32)
NB = 1160

NY = C * O * P        # qty (bf16) [64, (c, o, ii)]

_nc_cache = {}


def _build_nc():
    nc = bacc.Bacc()
    big_d = nc.dram_tensor("big", [P, NB], BF16, kind="ExternalInput")
    qty_d = nc.dram_tensor("qty", [2 * H, NY], BF16, kind="ExternalInput")
    y_d = nc.dram_tensor("y", [1, O, P], F32, kind="ExternalOutput")

    AxX = mybir.AxisListType.X
    IsGe = mybir.AluOpType.is_ge

    with tile.TileContext(nc) as tc:
        with (
            tc.tile_pool(name="sb", bufs=1) as sb,
            tc.tile_pool(name="ps", bufs=1, space="PSUM") as ps,
        ):
            big = sb.tile([P, NB], BF16)
            qty = sb.tile([2 * H, NY], BF16)
            nc.sync.dma_start(out=big[:], in_=big_d[:])
            nc.scalar.dma_start(out=qty[:], in_=qty_d[:])

            # ---- input-independent prep on POOL, free under the DMAs ----
            ones = sb.tile([P, 1], BF16)
            nc.gpsimd.memset(ones[:], 1.0)
            ut = sb.tile([P, P], BF16)
            # keep 1.0 where ii - jj >= 0 (causal), else 0
            nc.gpsimd.affine_select(ut[:], ones[:].broadcast_to([P, P]),
                                    pattern=[[1, P]], compare_op=IsGe,
                                    fill=0.0, base=0, channel_multiplier=-1)

            tw = big[:, TW_OFF:TW_OFF + 2 * H].rearrange(
                "p (s h) -> p s h", s=2)
            xwin = big[:, XW_OFF:XW_OFF + C]
            xm = big[:, XM_OFF:XM_OFF + M * C].rearrange(
                "p (b c) -> p b c", b=M)
            tt = big[:, TT_OFF:TT_OFF + 2 * M * H].rearrange(
                "p (b s h) -> p b s h", b=M, s=2)
            qt = big[:, QTS_OFF:QTS_OFF + 2 * O * P].rearrange(
                "p (s o i) -> p s o i", s=2, o=O)   # [:,0]=sin, [:,1]=cos
            xw2 = big[:, XW2_OFF:XW2_OFF + C]
            xm2 = big[:, XM2_OFF:XM2_OFF + M * C]
            qty_v = qty[:].rearrange("p (c n) -> p c n", c=C)

            # ---- window products R[jj, (s, c, h)] = TW[jj,s,h]*xwin[jj,c]
            R = sb.tile([P, 2, C, H], BF16)
            xw_b = xwin.unsqueeze(2).broadcast_to([P, C, H])
            for s in range(2):
                tw_b = tw[:, s].unsqueeze(1).broadcast_to([P, C, H])
                nc.vector.tensor_mul(R[:, s], tw_b, xw_b)

            # ---- b2-scaled row sums for the bias term
            srow = sb.tile([P, 2], BF16)
            with nc.allow_low_precision(reason="4/32-term bf16 row sums"):
                nc.vector.reduce_sum(srow[:, 0:1], xw2, axis=AxX)
                nc.vector.reduce_sum(srow[:, 1:2], xm2, axis=AxX)

            # ---- PE contractions ----
            # windowed causal prefixes (first: they gate the G quarters)
            pwS = ps.tile([P, P], F32)
            pwC = ps.tile([P, P], F32)
            nc.tensor.matmul(pwS[:], R[:, 0], ut[:], start=True, stop=True)
            nc.tensor.matmul(pwC[:], R[:, 1], ut[:], start=True, stop=True)
            # block prefix: colT[(s,h), c] = sum_b TT_b.T @ xm_b
            colT = ps.tile([2 * H, C], F32)
            for b in range(M):
                nc.tensor.matmul(colT[:], tt[:, b], xm[:, b, :],
                                 start=(b == 0), stop=(b == M - 1))
            # bias prefix: pwx[0, ii] = b2 * (window prefix + block sum) of x
            pwx = ps.tile([1, P], F32)
            nc.tensor.matmul(pwx[:], srow[:, 0:1], ut[:],
                             start=True, stop=False)
            nc.tensor.matmul(pwx[:], srow[:, 1:2],
                             ones[:].broadcast_to([P, P]),
                             start=False, stop=True)

            # ---- drain colT/pwx from PSUM on the (idle) ACT engine ----
            colT_sb = sb.tile([2 * H, C], BF16)
            nc.scalar.copy(colT_sb[:], colT[:])
            pwx_sb = sb.tile([1, P], F32)
            nc.scalar.copy(pwx_sb[:], pwx[:])

            # ---- final projection: one PSUM accumulation group ----
            # y1 = sum_c colT_c.T @ QTY_c + sum_p pw_s * QT'_so
            y1 = ps.tile([1, O, P], F32)
            for c in range(C):
                nc.tensor.matmul(y1[:], colT_sb[:, c:c + 1], qty_v[:, c],
                                 start=(c == 0), stop=False,
                                 skip_group_check=True)
            # G quarters pipelined DVE -> PE (pw_s pairs with opposite trig)
            G = sb.tile([P, 2, O, P], BF16)
            pw = [pwS, pwC]
            for k, (s, o) in enumerate([(0, 0), (1, 0), (0, 1), (1, 1)]):
                nc.vector.tensor_mul(G[:, s, o], pw[s][:], qt[:, 1 - s, o])
                nc.tensor.matmul(y1[:, o], ones[:], G[:, s, o],
                                 start=False, stop=(k == 3),
                                 skip_group_check=True)

            # ---- y = pwx + y1 ----
            ysb = sb.tile([1, O, P], F32)
            pwx_b = pwx_sb[:].unsqueeze(1).broadcast_to([1, O, P])
            nc.vector.tensor_add(ysb[:], pwx_b, y1[:])
            nc.sync.dma_start(out=y_d[:], in_=ysb[:])
    nc.finalize()
    return nc


def _host_inputs(x, w1, b1, w2, b2):
    """Per-core input maps.  Host precomputes all weight-only trig."""
    x = np.asarray(x, np.float64)
    w1 = np.asarray(w1, np.float64)
    b1 = np.asarray(b1, np.float64)
    w2 = np.asarray(w2, np.float64)
    b2 = np.asarray(b2, np.float64)

    A = w1[:, 0] / T                                   # [H]
    jj = np.arange(P)
    bb = np.arange(M)
    ang = A[None, None, :] * (P * bb[None, :, None] + jj[:, None, None])
    ttfull = np.stack([np.sin(ang), np.cos(ang)], axis=1)  # [jj, s, b, h]
    cc = np.arange(C)
    oo = np.arange(O)
    off = (oo[:, None, None] * w1[:, 2]
           + cc[None, :, None] * w1[:, 1] + b1)        # [o, c, h]

    xr = x.reshape(M, P, C)
    in_maps = []
    for m in range(M):
        i_vals = P * m + jj                            # [ii]
        q = off[:, :, :, None] - A[None, None, :, None] * i_vals  # [o,c,h,ii]
        qts = (w2[0][None, :, None, None] * np.sin(q).transpose(1, 2, 0, 3)
               ).reshape(P, O * P)                     # [p=(c,h), (o,ii)]
        qtc = (w2[0][None, :, None, None] * np.cos(q).transpose(1, 2, 0, 3)
               ).reshape(P, O * P)
        xmask = x.copy()
        xmask[P * m:] = 0.0
        xm = xmask.reshape(M, P, C).transpose(1, 0, 2).reshape(P, M * C)

        big = np.zeros((P, NB), np.float64)
        big[:, TW_OFF:TW_OFF + 2 * H] = ttfull[:, :, m, :].reshape(P, 2 * H)
        big[:, XW_OFF:XW_OFF + C] = xr[m]
        big[:, XM_OFF:XM_OFF + M * C] = xm
        big[:, TT_OFF:TT_OFF + 2 * M * H] = ttfull.transpose(
            0, 2, 1, 3).reshape(P, 2 * M * H)          # [jj, (b, s, h)]
        big[:, QTS_OFF:QTS_OFF + O * P] = qts
        big[:, QTC_OFF:QTC_OFF + O * P] = qtc
        big[:, XW2_OFF:XW2_OFF + C] = b2[0] * xr[m]
        big[:, XM2_OFF:XM2_OFF + M * C] = b2[0] * xm

        # qty[(s', h), (c, o, ii)]: s'=0 pairs col_s with QTc', s'=1 with QTs'
        qtyf = np.empty((2 * H, C, O * P), np.float64)
        qtyf[0:H] = qtc.reshape(C, H, O * P).transpose(1, 0, 2)
        qtyf[H:2 * H] = qts.reshape(C, H, O * P).transpose(1, 0, 2)
        qtyf = qtyf.reshape(2 * H, NY)

        in_maps.append({
            "big": big.astype(ml_dtypes.bfloat16),
            "qty": qtyf.astype(ml_dtypes.bfloat16),
        })
    return in_maps


def kernel(x, t, w1, b1, w2, b2, out_channels):
    if "nc" not in _nc_cache:
        _nc_cache["nc"] = _build_nc()
    nc = _nc_cache["nc"]
    in_maps = _host_inputs(x, w1, b1, w2, b2)
    res = run_bass_kernel_spmd(nc, in_maps, core_ids=list(range(M)))
    y = np.empty((T, O), np.float32)
    for m in range(M):
        ym = np.asarray(res.results[m]["y"]).reshape(O, P)
        y[P * m:P * (m + 1), :] = ym.T
    return y


# revision 21
# speedup vs baseline: 1.1240x; 1.0010x over previous
"""Trainium2 Bass kernel for nn_CkConv1D (continuous-kernel causal conv).

Math: the reference builds a T x T Toeplitz kernel K[o,c,i,j] =
sum_h w2[h]*sin(A_h*(j-i) + off[o,c,h]) + b2  (A_h = w1[h,0]/T), masks it
causally (j<=i) and contracts with x.  Using sin(X+Y) = sinX cosY + cosX sinY
with X = A_h*j, Y = off - A_h*i, the masked contraction factorizes into
causal prefix sums over j of sin(A_h j)x[j,c] / cos(A_h j)x[j,c].

Work is sharded over 8 NeuronCores: core m produces output rows
[128m, 128m+128).  The host precomputes every weight-only quantity (the
trig basis over j, the per-core window trig, the w2-scaled query-side
trig) so the device does only the x-dependent contractions:

  ut[jj,ii]       = causal mask, built by POOL affine_select
                    while the input DMAs are still in flight
  R[jj,(s,c,h)]   = TW[jj,s,h] * xwin[jj,c]                  (DVE)
  pwS/pwC[p,ii]   = R_s.T @ ut   (windowed causal prefix)    (PE, bf16)
  colT[(s,h),c]   = sum_b TT_b.T @ xm_b  (block prefix)      (PE, bf16)
  G_so            = pw_s * QT_o'  (4 quarters, pipelined     (DVE)
  y1              = sum_c colT_c.T @ QTY_c                    with PE)
                    + ones.T @ G_so   (one PSUM accumulation group)
  y               = y1 + causal prefix of b2*sum_c x

The block-prefix correction never needs a partition transpose: its
contribution to y is linear in colT, so it folds into the final PSUM
accumulation as four rank-1 matmuls against the host-built QTY grid.
The DMA phase is aggregate-HBM-bound across the 8 cores, so all device
constants travel in a single bf16 tensor (plus QTY), and b2 rides as
pre-scaled copies of the x row sums instead of an fp32 scalar.

Partition layout: p = c*32 + h (C_in=4 channels x H=32 hidden = 128).
The program is identical on every core (SPMD); per-core behavior comes
only from per-core input data.
"""

import sys
from pathlib import Path

import numpy as np

for _p in ("/opt/trn_rl_repo",):
    if _p not in sys.path and Path(_p).exists():
        sys.path.insert(0, _p)

import ml_dtypes
import concourse.bass as bass
import concourse.bacc as bacc
import concourse.tile as tile
from concourse import mybir
from concourse.bass_utils import run_bass_kernel_spmd

F32 = mybir.dt.float32
BF16 = mybir.dt.bfloat16
T, C, O, H, P, M = 1024, 4, 2, 32, 128, 8

# big (bf16) column offsets — compute-side data, needed first
TW_OFF = 0            # window trig [jj, (s, h)]            (64)
XW_OFF = 64           # window x  [jj, c]                   (4)
XM_OFF = 68           # masked x  [jj, (b, c)]              (32)
TT_OFF = 100          # basis trig [jj, (b, s, h)]          (512)
XW2_OFF = 612         # b2 * xwin                           (4)
XM2_OFF = 616         # b2 * xm                             (32)
NB = 648

# qtg (bf16): query-side trig, needed last -> slow SWDGE queue
QTS_OFF = 0           # w2[h]*sin(off - A_h*i) [p, (o,ii)]  (256)
QTC_OFF = 256         # w2[h]*cos(off - A_h*i) [p, (o,ii)]  (256)
NQ = 512

NY = C * O * P        # qty (bf16) [64, (c, o, ii)]

_nc_cache = {}


def _build_nc():
    nc = bacc.Bacc()
    big_d = nc.dram_tensor("big", [P, NB], BF16, kind="ExternalInput")
    qty_d = nc.dram_tensor("qty", [2 * H, NY], BF16, kind="ExternalInput")
    qtg_d = nc.dram_tensor("qtg", [P, NQ], BF16, kind="ExternalInput")
    y_d = nc.dram_tensor("y", [1, O, P], F32, kind="ExternalOutput")

    AxX = mybir.AxisListType.X
    IsGe = mybir.AluOpType.is_ge

    with tile.TileContext(nc) as tc:
        with (
            tc.tile_pool(name="sb", bufs=1) as sb,
            tc.tile_pool(name="ps", bufs=1, space="PSUM") as ps,
        ):
            big = sb.tile([P, NB], BF16)
            qty = sb.tile([2 * H, NY], BF16)
            qtg = sb.tile([P, NQ], BF16)
            nc.sync.dma_start(out=big[:], in_=big_d[:])
            nc.scalar.dma_start(out=qty[:], in_=qty_d[:])
            nc.gpsimd.dma_start(out=qtg[:], in_=qtg_d[:])

            # ---- input-independent prep on POOL, free under the DMAs ----
            ones = sb.tile([P, 1], BF16)
            nc.gpsimd.memset(ones[:], 1.0)
            ut = sb.tile([P, P], BF16)
            # keep 1.0 where ii - jj >= 0 (causal), else 0
            nc.gpsimd.affine_select(ut[:], ones[:].broadcast_to([P, P]),
                                    pattern=[[1, P]], compare_op=IsGe,
                                    fill=0.0, base=0, channel_multiplier=-1)

            tw = big[:, TW_OFF:TW_OFF + 2 * H].rearrange(
                "p (s h) -> p s h", s=2)
            xwin = big[:, XW_OFF:XW_OFF + C]
            xm = big[:, XM_OFF:XM_OFF + M * C].rearrange(
                "p (b c) -> p b c", b=M)
            tt = big[:, TT_OFF:TT_OFF + 2 * M * H].rearrange(
                "p (b s h) -> p b s h", b=M, s=2)
            qt = qtg[:, QTS_OFF:QTS_OFF + 2 * O * P].rearrange(
                "p (s o i) -> p s o i", s=2, o=O)   # [:,0]=sin, [:,1]=cos
            xw2 = big[:, XW2_OFF:XW2_OFF + C]
            xm2 = big[:, XM2_OFF:XM2_OFF + M * C]
            qty_v = qty[:].rearrange("p (c n) -> p c n", c=C)

            # ---- window products R[jj, (s, c, h)] = TW[jj,s,h]*xwin[jj,c]
            R = sb.tile([P, 2, C, H], BF16)
            xw_b = xwin.unsqueeze(2).broadcast_to([P, C, H])
            for s in range(2):
                tw_b = tw[:, s].unsqueeze(1).broadcast_to([P, C, H])
                nc.vector.tensor_mul(R[:, s], tw_b, xw_b)

            # ---- b2-scaled row sums for the bias term
            srow = sb.tile([P, 2], BF16)
            with nc.allow_low_precision(reason="4/32-term bf16 row sums"):
                nc.vector.reduce_sum(srow[:, 0:1], xw2, axis=AxX)
                nc.vector.reduce_sum(srow[:, 1:2], xm2, axis=AxX)

            # ---- PE contractions ----
            # windowed causal prefixes (first: they gate the G quarters)
            pwS = ps.tile([P, P], F32)
            pwC = ps.tile([P, P], F32)
            nc.tensor.matmul(pwS[:], R[:, 0], ut[:], start=True, stop=True)
            nc.tensor.matmul(pwC[:], R[:, 1], ut[:], start=True, stop=True)
            # block prefix: colT[(s,h), c] = sum_b TT_b.T @ xm_b
            colT = ps.tile([2 * H, C], F32)
            for b in range(M):
                nc.tensor.matmul(colT[:], tt[:, b], xm[:, b, :],
                                 start=(b == 0), stop=(b == M - 1))
            # bias prefix: pwx[0, ii] = b2 * (window prefix + block sum) of x
            pwx = ps.tile([1, P], F32)
            nc.tensor.matmul(pwx[:], srow[:, 0:1], ut[:],
                             start=True, stop=False)
            nc.tensor.matmul(pwx[:], srow[:, 1:2],
                             ones[:].broadcast_to([P, P]),
                             start=False, stop=True)

            # ---- drain colT/pwx from PSUM on the (idle) ACT engine ----
            colT_sb = sb.tile([2 * H, C], BF16)
            nc.scalar.copy(colT_sb[:], colT[:])
            pwx_sb = sb.tile([1, P], F32)
            nc.scalar.copy(pwx_sb[:], pwx[:])

            # ---- final projection: one PSUM accumulation group ----
            # y1 = sum_c colT_c.T @ QTY_c + sum_p pw_s * QT'_so
            y1 = ps.tile([1, O, P], F32)
            for c in range(C):
                nc.tensor.matmul(y1[:], colT_sb[:, c:c + 1], qty_v[:, c],
                                 start=(c == 0), stop=False,
                                 skip_group_check=True)
            # G quarters pipelined DVE -> PE (pw_s pairs with opposite trig)
            G = sb.tile([P, 2, O, P], BF16)
            pw = [pwS, pwC]
            for k, (s, o) in enumerate([(0, 0), (1, 0), (0, 1), (1, 1)]):
                nc.vector.tensor_mul(G[:, s, o], pw[s][:], qt[:, 1 - s, o])
                nc.tensor.matmul(y1[:, o], ones[:], G[:, s, o],
                                 start=False, stop=(k == 3),
                                 skip_group_check=True)

            # ---- y = pwx + y1 ----
            ysb = sb.tile([1, O, P], F32)
            pwx_b = pwx_sb[:].unsqueeze(1).broadcast_to([1, O, P])
            nc.vector.tensor_add(ysb[:], pwx_b, y1[:])
            nc.sync.dma_start(out=y_d[:], in_=ysb[:])
    nc.finalize()
    return nc


def _host_inputs(x, w1, b1, w2, b2):
    """Per-core input maps.  Host precomputes all weight-only trig."""
    x = np.asarray(x, np.float64)
    w1 = np.asarray(w1, np.float64)
    b1 = np.asarray(b1, np.float64)
    w2 = np.asarray(w2, np.float64)
    b2 = np.asarray(b2, np.float64)

    A = w1[:, 0] / T                                   # [H]
    jj = np.arange(P)
    bb = np.arange(M)
    ang = A[None, None, :] * (P * bb[None, :, None] + jj[:, None, None])
    ttfull = np.stack([np.sin(ang), np.cos(ang)], axis=1)  # [jj, s, b, h]
    cc = np.arange(C)
    oo = np.arange(O)
    off = (oo[:, None, None] * w1[:, 2]
           + cc[None, :, None] * w1[:, 1] + b1)        # [o, c, h]

    xr = x.reshape(M, P, C)
    in_maps = []
    for m in range(M):
        i_vals = P * m + jj                            # [ii]
        q = off[:, :, :, None] - A[None, None, :, None] * i_vals  # [o,c,h,ii]
        qts = (w2[0][None, :, None, None] * np.sin(q).transpose(1, 2, 0, 3)
               ).reshape(P, O * P)                     # [p=(c,h), (o,ii)]
        qtc = (w2[0][None, :, None, None] * np.cos(q).transpose(1, 2, 0, 3)
               ).reshape(P, O * P)
        xmask = x.copy()
        xmask[P * m:] = 0.0
        xm = xmask.reshape(M, P, C).transpose(1, 0, 2).reshape(P, M * C)

        big = np.zeros((P, NB), np.float64)
        big[:, TW_OFF:TW_OFF + 2 * H] = ttfull[:, :, m, :].reshape(P, 2 * H)
        big[:, XW_OFF:XW_OFF + C] = xr[m]
        big[:, XM_OFF:XM_OFF + M * C] = xm
        big[:, TT_OFF:TT_OFF + 2 * M * H] = ttfull.transpose(
            0, 2, 1, 3).reshape(P, 2 * M * H)          # [jj, (b, s, h)]
        big[:, XW2_OFF:XW2_OFF + C] = b2[0] * xr[m]
        big[:, XM2_OFF:XM2_OFF + M * C] = b2[0] * xm

        qtg = np.zeros((P, NQ), np.float64)
        qtg[:, QTS_OFF:QTS_OFF + O * P] = qts
        qtg[:, QTC_OFF:QTC_OFF + O * P] = qtc

        # qty[(s', h), (c, o, ii)]: s'=0 pairs col_s with QTc', s'=1 with QTs'
        qtyf = np.empty((2 * H, C, O * P), np.float64)
        qtyf[0:H] = qtc.reshape(C, H, O * P).transpose(1, 0, 2)
        qtyf[H:2 * H] = qts.reshape(C, H, O * P).transpose(1, 0, 2)
        qtyf = qtyf.reshape(2 * H, NY)

        in_maps.append({
            "big": big.astype(ml_dtypes.bfloat16),
            "qty": qtyf.astype(ml_dtypes.bfloat16),
            "qtg": qtg.astype(ml_dtypes.bfloat16),
        })
    return in_maps


def kernel(x, t, w1, b1, w2, b2, out_channels):
    if "nc" not in _nc_cache:
        _nc_cache["nc"] = _build_nc()
    nc = _nc_cache["nc"]
    in_maps = _host_inputs(x, w1, b1, w2, b2)
    res = run_bass_kernel_spmd(nc, in_maps, core_ids=list(range(M)))
    y = np.empty((T, O), np.float32)
    for m in range(M):
        ym = np.asarray(res.results[m]["y"]).reshape(O, P)
        y[P * m:P * (m + 1), :] = ym.T
    return y
